# revision 6
# baseline (speedup 1.0000x reference)
"""ClusterNet (vq_codebook) kernel for 8x Trainium2 NeuronCores (Bass/Tile).

Reference math (ALPHA = 1):
    d2   = |z - c|^2                     z: (8192, 2048)  c: (512, 2048)
    Qun  = (1 + sqrt(d2))^-1
    Q    = Qun / rowsum(Qun)
    P    = (Q^2 / colsum(Q)) / rowsum(Q^2 / colsum(Q))
    out  = stack([Q, P])                 (2, 8192, 512) float32

Distribution: data-parallel over the batch - 1024 rows per core, centroids
replicated. Cross-core communication: the per-core column-sum of Q (512
floats), exchanged with two AllGathers (one per 512-row half so the first
hides under the second half's matmuls); each core then reduces the [16,512]
gather locally with a ones-matvec on the PE.

A dummy AllGather is fired at kernel start and gates the input DMAs: it
absorbs the CC-stream init barrier + first-op ncfw latency and aligns the
cores (the runtime staggers core kickoff by 10-30us; without the gate every
core burns that stagger waiting at the first real collective).

Per-core pipeline (8 m-tiles of 128 rows):
  PE   : S = d2 - 1 accumulated in PSUM per m-tile: one bf16 affine matmul
         (K=4: rows carrying csq_hi-1/csq_lo/zsq_hi/zsq_lo) + 8 fp8
         DoubleRow matmuls (K=256 each) over the 2048-dim contraction.
  ACT  : u = Sqrt(S + 1) = sqrt(d2) straight out of PSUM.
  DVE  : r1 = recip_approx(S) from PSUM; Qun = (u - 1) * r1 with fused
         row-accumulate (identity: 1/(1+u) = (u-1)/(u^2-1)); rq = 1/rowsum.
  PE   : colsum of Q per half via matvec lhsT=rq, rhs=Qun -> [1,512] PSUM
         (emitted one m-tile late so the strict-FIFO PE queue never stalls
         on the DVE chain).
  CC   : AllGather the [1,512] halves -> [8,512]; ones-matvec sums them.
  PE   : outer-product ones x (1/s) broadcasts 1/s into PSUM [128,512].
  ACT  : q = Qun*rq (Copy w/ per-partition scale, bf16); q2 = Qun^2 (bf16).
  DVE  : W = q2 * rinv (STT, in1 straight from PSUM, fused row-accumulate);
         rw = 1/ws.
  ACT  : p = W*rw (Copy w/ scale, bf16).

Outputs are staged bf16, partition-major [128, 8, 512]; the host transposes
to [1024, 512] and upcasts to fp32.
"""

import sys

import numpy as np

if "/opt/trn_rl_repo" not in sys.path:
    sys.path.insert(0, "/opt/trn_rl_repo")

import ml_dtypes

import concourse.bacc as bacc
import concourse.mybir as mybir
import concourse.tile as tile
from concourse.bass_utils import run_bass_kernel_spmd

BF16 = ml_dtypes.bfloat16
FP8 = ml_dtypes.float8_e4m3

N_CORES = 8
BS, NH, NC_CLUST = 8192, 2048, 512
B_CORE = BS // N_CORES          # 1024 rows per core
M_TILES = B_CORE // 128         # 8
KC = 8                          # DoubleRow k-chunks (256 contraction each)
KX = 4                          # affine rows: csq_hi-1, csq_lo, zsq_hi, zsq_lo

_nc_cache = None


def _build_nc():
    F = mybir.ActivationFunctionType
    A = mybir.AluOpType
    f32 = mybir.dt.float32
    bf16 = mybir.dt.bfloat16
    fp8 = mybir.dt.float8e4
    DR = mybir.MatmulPerfMode.DoubleRow

    nc = bacc.Bacc("TRN2", target_bir_lowering=False, debug=False,
                   num_devices=N_CORES)
    # z^T fp8 per m-tile: [m, p, kc, j, r] = z[m*128+r, h], h=(kc*2+j)*128+p.
    ztd = nc.dram_tensor("zt", [M_TILES, 128, KC * 2 * 128], fp8,
                         kind="ExternalInput")
    # -2*c^T fp8 in two kc-halves: [h2, p, kcl, j, c]. Line = 4KB.
    ctd = nc.dram_tensor("ct", [2, 128, 4 * 2 * NC_CLUST], fp8,
                         kind="ExternalInput")
    zxd = nc.dram_tensor("zx", [KX, B_CORE], bf16, kind="ExternalInput")
    cxd = nc.dram_tensor("cx", [KX, NC_CLUST], bf16, kind="ExternalInput")
    q_out = nc.dram_tensor("q", [128, M_TILES, NC_CLUST], bf16,
                           kind="ExternalOutput")
    p_out = nc.dram_tensor("p", [128, M_TILES, NC_CLUST], bf16,
                           kind="ExternalOutput")

    groups = [list(range(N_CORES))]

    with tile.TileContext(nc) as tc:
        with (
            tc.tile_pool(name="zin", bufs=1) as zin,
            tc.tile_pool(name="cin", bufs=1) as cin,
            tc.tile_pool(name="upool", bufs=2) as upool,
            tc.tile_pool(name="rpool", bufs=2) as rpool,
            tc.tile_pool(name="wpool", bufs=3) as wpool,
            tc.tile_pool(name="work", bufs=1) as work,
            tc.tile_pool(name="small", bufs=1) as small,
            tc.tile_pool(name="psum", bufs=4, space="PSUM") as psum,
            tc.tile_pool(name="cpsum", bufs=1, space="PSUM") as cpsum,
            tc.tile_pool(name="dram", bufs=1, space="DRAM") as dram,
        ):
            # ---- warm-up / alignment collective, fired immediately ----
            dum_sb = small.tile([1, 8], f32, tag="dum")
            nc.vector.memset(dum_sb, 0.0)
            dum_in = dram.tile([1, 8], f32, name="dum_in")
            dum_out = dram.tile([8, 8], f32, name="dum_out")
            nc.sync.dma_start(out=dum_in[:, :], in_=dum_sb)
            nc.gpsimd.collective_compute(
                "AllGather", A.bypass, replica_groups=groups,
                ins=[dum_in.opt()], outs=[dum_out.opt()],
            )
            gate_sb = small.tile([8, 8], f32, tag="gate")
            gate = nc.sync.dma_start(out=gate_sb, in_=dum_out[:, :])

            # ---- input DMA, gated on the alignment collective ----
            def gated_dma(out, in_):
                inst = nc.sync.dma_start(out=out, in_=in_)
                tile.add_dep_helper(inst.ins, gate.ins, sync=True,
                                    reason="start-alignment gate")
                return inst

            zx = zin.tile([KX, B_CORE], bf16, tag="zx")
            gated_dma(zx, zxd.ap())
            cx = cin.tile([KX, NC_CLUST], bf16, tag="cx")
            gated_dma(cx, cxd.ap())
            ct2 = []
            for h in range(2):
                t = cin.tile([128, 4, 2, NC_CLUST], fp8, tag=f"ct{h}",
                             name=f"ct2_{h}")
                ct2.append(t)
            ztm = []
            for m in range(M_TILES):
                t = zin.tile([128, KC, 2, 128], fp8, tag=f"zt{m}",
                             name=f"ztm{m}")
                ztm.append(t)
            gated_dma(ct2[0], ctd.ap()[0].rearrange(
                "p (k j c) -> p k j c", k=4, j=2))
            gated_dma(ztm[0], ztd.ap()[0].rearrange(
                "p (k j r) -> p k j r", k=KC, j=2))
            gated_dma(ztm[1], ztd.ap()[1].rearrange(
                "p (k j r) -> p k j r", k=KC, j=2))
            gated_dma(ct2[1], ctd.ap()[1].rearrange(
                "p (k j c) -> p k j c", k=4, j=2))
            for m in range(2, M_TILES):
                gated_dma(ztm[m], ztd.ap()[m].rearrange(
                    "p (k j r) -> p k j r", k=KC, j=2))

            # ---- workspaces ----
            qun_all = work.tile([128, M_TILES, NC_CLUST], f32, tag="qun")
            q2_all = work.tile([128, M_TILES, NC_CLUST], bf16, tag="q2")
            qbuf = work.tile([128, M_TILES, NC_CLUST], bf16, tag="qb")
            sq_all = small.tile([128, M_TILES], f32, tag="sq")
            rq_all = small.tile([128, M_TILES], f32, tag="rq")
            ws_all = small.tile([128, M_TILES], f32, tag="ws")
            rw_all = small.tile([128, M_TILES], f32, tag="rw")
            cs_sb = [small.tile([1, NC_CLUST], f32, tag=f"cs{h}",
                                name=f"cs_sb{h}")
                     for h in range(2)]
            agsb = small.tile([16, NC_CLUST], f32, tag="ag")
            agbf = small.tile([16, NC_CLUST], bf16, tag="agbf")
            rinv_sb = small.tile([1, NC_CLUST], f32, tag="rinv")
            rinv_bf = small.tile([1, NC_CLUST], bf16, tag="rinvbf")
            ones16 = small.tile([16, 1], bf16, tag="o16")
            nc.vector.memset(ones16, 1.0)
            ones128 = small.tile([1, 128], bf16, tag="o128")
            nc.vector.memset(ones128, 1.0)
            cc_in = [dram.tile([1, NC_CLUST], f32, name=f"cc_in{h}")
                     for h in range(2)]
            cc_out = [dram.tile([8, NC_CLUST], f32, name=f"cc_out{h}")
                      for h in range(2)]

            cps = [cpsum.tile([1, NC_CLUST], f32, tag=f"cps{h}",
                              name=f"cps{h}")
                   for h in range(2)]

            # ---- main stream: per m-tile MMs + Q chain ----
            # The cps matvec for tile m is emitted after tile m+1's matmuls
            # so the PE queue never waits on the DVE chain.
            def colsum_matvec(m):
                h = m // 4
                nc.tensor.matmul(cps[h], lhsT=rq_all[:, m:m + 1],
                                 rhs=qun_all[:, m, :],
                                 start=(m % 4 == 0), stop=(m % 4 == 3))
                if m % 4 == 3:
                    nc.vector.tensor_copy(cs_sb[h], cps[h])
                    nc.sync.dma_start(out=cc_in[h][:, :], in_=cs_sb[h])
                    nc.gpsimd.collective_compute(
                        "AllGather", A.bypass, replica_groups=groups,
                        ins=[cc_in[h].opt()], outs=[cc_out[h].opt()],
                    )

            for m in range(M_TILES):
                ps = psum.tile([128, NC_CLUST], f32, tag="mm")
                nc.tensor.matmul(ps, lhsT=zx[:, m * 128:(m + 1) * 128],
                                 rhs=cx, start=True, stop=False)
                for kc in range(KC):
                    h2, kcl = divmod(kc, 4)
                    nc.tensor.matmul(
                        ps, lhsT=ztm[m][:, kc, :, :],
                        rhs=ct2[h2][:, kcl, :, :],
                        start=False, stop=(kc == KC - 1), perf_mode=DR)
                if m >= 1:
                    colsum_matvec(m - 1)

                u = upool.tile([128, NC_CLUST], f32, tag="u")
                nc.scalar.activation(u, ps, F.Sqrt, bias=1.0)
                r1 = rpool.tile([128, NC_CLUST], f32, tag="r1")
                nc.vector.reciprocal_approx_fast(out=r1, in_=ps)
                qun = qun_all[:, m, :]
                nc.vector.scalar_tensor_tensor(
                    out=qun, in0=u, scalar=-1.0, in1=r1,
                    op0=A.add, op1=A.mult,
                    accum_out=sq_all[:, m:m + 1])
                nc.vector.reciprocal(rq_all[:, m:m + 1], sq_all[:, m:m + 1])
                # under-collective work (ACT): q2 and q output
                if m >= 1:
                    mp = m - 1
                    nc.scalar.activation(q2_all[:, mp, :],
                                         qun_all[:, mp, :], F.Square)
                    nc.scalar.activation(qbuf[:, mp, :], qun_all[:, mp, :],
                                         F.Copy, scale=rq_all[:, mp:mp + 1])
            colsum_matvec(M_TILES - 1)

            mp = M_TILES - 1
            nc.scalar.activation(q2_all[:, mp, :], qun_all[:, mp, :],
                                 F.Square)
            nc.scalar.activation(qbuf[:, mp, :], qun_all[:, mp, :],
                                 F.Copy, scale=rq_all[:, mp:mp + 1])
            nc.sync.dma_start(out=q_out.ap(), in_=qbuf)

            # ---- post-collective: s -> 1/s broadcast -> P ----
            nc.sync.dma_start(out=agsb[0:8, :], in_=cc_out[0][:, :])
            nc.sync.dma_start(out=agsb[8:16, :], in_=cc_out[1][:, :])
            nc.scalar.activation(agbf, agsb, F.Copy)
            ssum = cpsum.tile([1, NC_CLUST], f32, tag="ss")
            nc.tensor.matmul(ssum, lhsT=ones16, rhs=agbf,
                             start=True, stop=True)
            nc.vector.reciprocal_approx_fast(out=rinv_sb, in_=ssum)
            nc.scalar.activation(rinv_bf, rinv_sb, F.Copy)
            bbc = cpsum.tile([128, NC_CLUST], f32, tag="bbc")
            nc.tensor.matmul(bbc, lhsT=ones128, rhs=rinv_bf,
                             start=True, stop=True)

            for m in range(M_TILES):
                w = wpool.tile([128, NC_CLUST], bf16, tag="w")
                nc.vector.scalar_tensor_tensor(
                    out=w, in0=q2_all[:, m, :], scalar=0.0, in1=bbc,
                    op0=A.bypass, op1=A.mult,
                    accum_out=ws_all[:, m:m + 1])
                nc.vector.reciprocal(rw_all[:, m:m + 1], ws_all[:, m:m + 1])
                pt = wpool.tile([128, NC_CLUST], bf16, tag="pt")
                nc.scalar.activation(pt, w, F.Copy,
                                     scale=rw_all[:, m:m + 1])
                nc.sync.dma_start(out=p_out.ap()[:, m, :], in_=pt)
    nc.compile()
    return nc


def _get_nc():
    global _nc_cache
    if _nc_cache is None:
        _nc_cache = _build_nc()
    return _nc_cache


def _split_hi_lo(x64):
    """Split float64 values into bf16 hi + bf16 lo with hi + lo ~= x."""
    hi = x64.astype(BF16)
    lo = (x64 - hi.astype(np.float64)).astype(BF16)
    return hi, lo


def _prep_inputs(z, centroids):
    z = np.asarray(z, dtype=np.float32)
    c = np.asarray(centroids, dtype=np.float32)

    csqm1 = np.sum(c.astype(np.float64) ** 2, axis=1) - 1.0   # (512,)
    csq_hi, csq_lo = _split_hi_lo(csqm1)
    cx = np.empty((KX, NC_CLUST), dtype=BF16)
    cx[0] = csq_hi
    cx[1] = csq_lo
    cx[2] = BF16(1.0)
    cx[3] = BF16(1.0)

    zsq = np.sum(z.astype(np.float64) ** 2, axis=1)           # (8192,)
    zsq_hi, zsq_lo = _split_hi_lo(zsq)

    # z^T fp8: [h, b] -> [kc, j, p, b]
    zT8 = z.T.reshape(KC, 2, 128, BS).astype(FP8)
    # -2c^T fp8: [h2, kcl, j, p, c] -> [h2, p, kcl, j, c]
    ct8 = np.ascontiguousarray(
        (-2.0 * c.T).reshape(2, 4, 2, 128, NC_CLUST)
        .transpose(0, 3, 1, 2, 4)).astype(FP8).reshape(2, 128, 4 * 2 * NC_CLUST)

    in_maps = []
    for core in range(N_CORES):
        s = slice(core * B_CORE, (core + 1) * B_CORE)
        zx = np.empty((KX, B_CORE), dtype=BF16)
        zx[0] = BF16(1.0)
        zx[1] = BF16(1.0)
        zx[2] = zsq_hi[s]
        zx[3] = zsq_lo[s]
        # [kc, j, p, 8m, 128r] -> [m, p, kc, j, r]
        zc = np.ascontiguousarray(
            zT8[:, :, :, s].reshape(KC, 2, 128, M_TILES, 128)
            .transpose(3, 2, 0, 1, 4)).reshape(M_TILES, 128, KC * 2 * 128)
        in_maps.append({"zt": zc, "ct": ct8, "zx": zx, "cx": cx})
    return in_maps


def run(z, centroids, trace=False, trace_cores=None):
    """Run on the 8 NeuronCores. Returns (out, BassKernelResults)."""
    nc = _get_nc()
    in_maps = _prep_inputs(z, centroids)
    res = run_bass_kernel_spmd(
        nc, in_maps, list(range(N_CORES)),
        trace=trace, trace_cores=trace_cores,
    )
    qs, ps = [], []
    for core in range(N_CORES):
        qs.append(res.results[core]["q"].transpose(1, 0, 2)
                  .reshape(B_CORE, NC_CLUST))
        ps.append(res.results[core]["p"].transpose(1, 0, 2)
                  .reshape(B_CORE, NC_CLUST))
    q = np.concatenate(qs, axis=0).astype(np.float32)
    p = np.concatenate(ps, axis=0).astype(np.float32)
    out = np.stack([q, p])
    return out, res


def kernel(z, centroids):
    out, _ = run(z, centroids)
    return out


# revision 13
# speedup vs baseline: 1.4880x; 1.4880x over previous
"""ClusterNet (vq_codebook) kernel for 8x Trainium2 NeuronCores (Bass/Tile).

Reference math (ALPHA = 1):
    d2   = |z - c|^2                     z: (8192, 2048)  c: (512, 2048)
    Qun  = (1 + sqrt(d2))^-1
    Q    = Qun / rowsum(Qun)
    P    = (Q^2 / colsum(Q)) / rowsum(Q^2 / colsum(Q))
    out  = stack([Q, P])                 (2, 8192, 512) float32

Distribution: data-parallel over the batch - 1024 rows per core, centroids
replicated. Cross-core communication: the per-core column-sum of Q (512
floats), exchanged with two AllGathers (one per 512-row half so the first
hides under the second half's matmuls); each core then reduces the [16,512]
gather locally with a ones-matvec on the PE.

A dummy AllGather is fired at kernel start and gates the input DMAs: it
absorbs the CC-stream init barrier + first-op ncfw latency and aligns the
cores (the runtime staggers core kickoff by 10-30us; without the gate every
core burns that stagger waiting at the first real collective).

Per-core pipeline (8 m-tiles of 128 rows):
  PE   : S = d2 - 1 accumulated in PSUM per m-tile: one bf16 affine matmul
         (K=4: rows carrying csq_hi-1/csq_lo/zsq_hi/zsq_lo) + 8 fp8
         DoubleRow matmuls (K=256 each) over the 2048-dim contraction.
  ACT  : u = Sqrt(S + 1) = sqrt(d2) straight out of PSUM.
  DVE  : r1 = recip_approx(S) from PSUM; Qun = (u - 1) * r1 with fused
         row-accumulate (identity: 1/(1+u) = (u-1)/(u^2-1)); rq = 1/rowsum.
  PE   : colsum of Q per half via matvec lhsT=rq, rhs=Qun -> [1,512] PSUM
         (emitted one m-tile late so the strict-FIFO PE queue never stalls
         on the DVE chain).
  CC   : AllGather the [1,512] halves -> [8,512]; ones-matvec sums them.
  PE   : outer-product ones x (1/s) broadcasts 1/s into PSUM [128,512].
  ACT  : q = Qun*rq (Copy w/ per-partition scale, bf16); q2 = Qun^2 (bf16).
  DVE  : W = q2 * rinv (STT, in1 straight from PSUM, fused row-accumulate);
         rw = 1/ws.
  ACT  : p = W*rw (Copy w/ scale, bf16).

Outputs are staged bf16, partition-major [128, 8, 512]; the host transposes
to [1024, 512] and upcasts to fp32.
"""

import sys

import numpy as np

if "/opt/trn_rl_repo" not in sys.path:
    sys.path.insert(0, "/opt/trn_rl_repo")

import ml_dtypes

import concourse.bacc as bacc
import concourse.mybir as mybir
import concourse.tile as tile
from concourse.bass_utils import run_bass_kernel_spmd

BF16 = ml_dtypes.bfloat16
FP8 = ml_dtypes.float8_e4m3

N_CORES = 8
BS, NH, NC_CLUST = 8192, 2048, 512
B_CORE = BS // N_CORES          # 1024 rows per core
M_TILES = B_CORE // 128         # 8
KC = 8                          # DoubleRow k-chunks (256 contraction each)
KX = 4                          # affine rows: csq_hi-1, csq_lo, zsq_hi, zsq_lo

_nc_cache = None


def _build_nc():
    F = mybir.ActivationFunctionType
    A = mybir.AluOpType
    f32 = mybir.dt.float32
    bf16 = mybir.dt.bfloat16
    fp8 = mybir.dt.float8e4
    DR = mybir.MatmulPerfMode.DoubleRow

    nc = bacc.Bacc("TRN2", target_bir_lowering=False, debug=False,
                   num_devices=N_CORES)
    # z^T fp8 per m-tile: [m, p, kc, j, r] = z[m*128+r, h], h=(kc*2+j)*128+p.
    ztd = nc.dram_tensor("zt", [M_TILES, 128, KC * 2 * 128], fp8,
                         kind="ExternalInput")
    # -2*c^T fp8 in two kc-halves: [h2, p, kcl, j, c]. Line = 4KB.
    ctd = nc.dram_tensor("ct", [2, 128, 4 * 2 * NC_CLUST], fp8,
                         kind="ExternalInput")
    zxd = nc.dram_tensor("zx", [KX, B_CORE], bf16, kind="ExternalInput")
    cxd = nc.dram_tensor("cx", [KX, NC_CLUST], bf16, kind="ExternalInput")
    q_out = nc.dram_tensor("q", [128, M_TILES, NC_CLUST], bf16,
                           kind="ExternalOutput")
    p_out = nc.dram_tensor("p", [128, M_TILES, NC_CLUST], bf16,
                           kind="ExternalOutput")

    groups = [list(range(N_CORES))]

    with tile.TileContext(nc) as tc:
        with (
            tc.tile_pool(name="zin", bufs=1) as zin,
            tc.tile_pool(name="cin", bufs=1) as cin,
            tc.tile_pool(name="upool", bufs=2) as upool,
            tc.tile_pool(name="rpool", bufs=2) as rpool,
            tc.tile_pool(name="wpool", bufs=3) as wpool,
            tc.tile_pool(name="work", bufs=1) as work,
            tc.tile_pool(name="small", bufs=1) as small,
            tc.tile_pool(name="psum", bufs=4, space="PSUM") as psum,
            tc.tile_pool(name="cpsum", bufs=1, space="PSUM") as cpsum,
            tc.tile_pool(name="dram", bufs=1, space="DRAM") as dram,
        ):
            # ---- warm-up / alignment collective, fired immediately ----
            dum_sb = small.tile([1, 8], f32, tag="dum")
            nc.vector.memset(dum_sb, 0.0)
            dum_in = dram.tile([1, 8], f32, name="dum_in")
            dum_out = dram.tile([8, 8], f32, name="dum_out")
            nc.sync.dma_start(out=dum_in[:, :], in_=dum_sb)
            nc.gpsimd.collective_compute(
                "AllGather", A.bypass, replica_groups=groups,
                ins=[dum_in.opt()], outs=[dum_out.opt()],
            )
            # ---- input DMA ----
            def gated_dma(out, in_):
                return nc.sync.dma_start(out=out, in_=in_)

            zx = zin.tile([KX, B_CORE], bf16, tag="zx")
            gated_dma(zx, zxd.ap())
            cx = cin.tile([KX, NC_CLUST], bf16, tag="cx")
            gated_dma(cx, cxd.ap())
            ct2 = []
            for h in range(2):
                t = cin.tile([128, 4, 2, NC_CLUST], fp8, tag=f"ct{h}",
                             name=f"ct2_{h}")
                ct2.append(t)
            ztm = []
            for m in range(M_TILES):
                t = zin.tile([128, KC, 2, 128], fp8, tag=f"zt{m}",
                             name=f"ztm{m}")
                ztm.append(t)
            gated_dma(ct2[0], ctd.ap()[0].rearrange(
                "p (k j c) -> p k j c", k=4, j=2))
            gated_dma(ztm[0], ztd.ap()[0].rearrange(
                "p (k j r) -> p k j r", k=KC, j=2))
            gated_dma(ztm[1], ztd.ap()[1].rearrange(
                "p (k j r) -> p k j r", k=KC, j=2))
            gated_dma(ct2[1], ctd.ap()[1].rearrange(
                "p (k j c) -> p k j c", k=4, j=2))
            for m in range(2, M_TILES):
                gated_dma(ztm[m], ztd.ap()[m].rearrange(
                    "p (k j r) -> p k j r", k=KC, j=2))

            # ---- workspaces ----
            qun_all = work.tile([128, M_TILES, NC_CLUST], f32, tag="qun")
            q2_all = work.tile([128, M_TILES, NC_CLUST], bf16, tag="q2")
            qbuf = work.tile([128, M_TILES, NC_CLUST], bf16, tag="qb")
            sq_all = small.tile([128, M_TILES], f32, tag="sq")
            rq_all = small.tile([128, M_TILES], f32, tag="rq")
            ws_all = small.tile([128, M_TILES], f32, tag="ws")
            rw_all = small.tile([128, M_TILES], f32, tag="rw")
            cs_sb = [small.tile([1, NC_CLUST], f32, tag=f"cs{h}",
                                name=f"cs_sb{h}")
                     for h in range(2)]
            agsb = small.tile([16, NC_CLUST], f32, tag="ag")
            agbf = small.tile([16, NC_CLUST], bf16, tag="agbf")
            rinv_sb = small.tile([1, NC_CLUST], f32, tag="rinv")
            rinv_bf = small.tile([1, NC_CLUST], bf16, tag="rinvbf")
            ones16 = small.tile([16, 1], bf16, tag="o16")
            nc.vector.memset(ones16, 1.0)
            ones128 = small.tile([1, 128], bf16, tag="o128")
            nc.vector.memset(ones128, 1.0)
            cc_in = dram.tile([2, NC_CLUST], f32, name="cc_in")
            cc_out = dram.tile([16, NC_CLUST], f32, name="cc_out")

            cps = [cpsum.tile([1, NC_CLUST], f32, tag=f"cps{h}",
                              name=f"cps{h}")
                   for h in range(2)]

            # ---- main stream: per m-tile MMs + Q chain ----
            # The cps matvec for tile m is emitted after tile m+1's matmuls
            # so the PE queue never waits on the DVE chain.
            def colsum_matvec(m):
                h = m // 4
                nc.tensor.matmul(cps[h], lhsT=rq_all[:, m:m + 1],
                                 rhs=qun_all[:, m, :],
                                 start=(m % 4 == 0), stop=(m % 4 == 3))
                if m % 4 == 3:
                    nc.vector.tensor_copy(cs_sb[h], cps[h])
                    nc.sync.dma_start(out=cc_in[h:h + 1, :], in_=cs_sb[h])
                if m == M_TILES - 1:
                    nc.gpsimd.collective_compute(
                        "AllGather", A.bypass, replica_groups=groups,
                        ins=[cc_in.opt()], outs=[cc_out.opt()],
                    )

            for m in range(M_TILES):
                ps = psum.tile([128, NC_CLUST], f32, tag="mm")
                nc.tensor.matmul(ps, lhsT=zx[:, m * 128:(m + 1) * 128],
                                 rhs=cx, start=True, stop=False)
                for kc in range(KC):
                    h2, kcl = divmod(kc, 4)
                    nc.tensor.matmul(
                        ps, lhsT=ztm[m][:, kc, :, :],
                        rhs=ct2[h2][:, kcl, :, :],
                        start=False, stop=(kc == KC - 1), perf_mode=DR)
                if m >= 1:
                    colsum_matvec(m - 1)

                u = upool.tile([128, NC_CLUST], f32, tag="u")
                nc.scalar.activation(u, ps, F.Sqrt, bias=1.0)
                r1 = rpool.tile([128, NC_CLUST], f32, tag="r1")
                nc.vector.reciprocal_approx_fast(out=r1, in_=ps)
                qun = qun_all[:, m, :]
                nc.vector.scalar_tensor_tensor(
                    out=qun, in0=u, scalar=-1.0, in1=r1,
                    op0=A.add, op1=A.mult,
                    accum_out=sq_all[:, m:m + 1])
                nc.vector.reciprocal(rq_all[:, m:m + 1], sq_all[:, m:m + 1])
                # under-collective work (ACT): q2 and q output
                if m >= 1:
                    mp = m - 1
                    nc.scalar.activation(q2_all[:, mp, :],
                                         qun_all[:, mp, :], F.Square)
                    nc.scalar.activation(qbuf[:, mp, :], qun_all[:, mp, :],
                                         F.Copy, scale=rq_all[:, mp:mp + 1])
            colsum_matvec(M_TILES - 1)

            mp = M_TILES - 1
            nc.scalar.activation(q2_all[:, mp, :], qun_all[:, mp, :],
                                 F.Square)
            nc.scalar.activation(qbuf[:, mp, :], qun_all[:, mp, :],
                                 F.Copy, scale=rq_all[:, mp:mp + 1])
            nc.sync.dma_start(out=q_out.ap(), in_=qbuf)

            # ---- post-collective: s -> 1/s broadcast -> P ----
            nc.sync.dma_start(out=agsb, in_=cc_out[:, :])
            nc.scalar.activation(agbf, agsb, F.Copy)
            ssum = cpsum.tile([1, NC_CLUST], f32, tag="ss")
            nc.tensor.matmul(ssum, lhsT=ones16, rhs=agbf,
                             start=True, stop=True)
            nc.vector.reciprocal_approx_fast(out=rinv_sb, in_=ssum)
            nc.scalar.activation(rinv_bf, rinv_sb, F.Copy)
            bbc = cpsum.tile([128, NC_CLUST], f32, tag="bbc")
            nc.tensor.matmul(bbc, lhsT=ones128, rhs=rinv_bf,
                             start=True, stop=True)

            for m in range(M_TILES):
                w = wpool.tile([128, NC_CLUST], bf16, tag="w")
                nc.vector.scalar_tensor_tensor(
                    out=w, in0=q2_all[:, m, :], scalar=0.0, in1=bbc,
                    op0=A.bypass, op1=A.mult,
                    accum_out=ws_all[:, m:m + 1])
                nc.vector.reciprocal(rw_all[:, m:m + 1], ws_all[:, m:m + 1])
                pt = wpool.tile([128, NC_CLUST], bf16, tag="pt")
                nc.scalar.activation(pt, w, F.Copy,
                                     scale=rw_all[:, m:m + 1])
                nc.sync.dma_start(out=p_out.ap()[:, m, :], in_=pt)
    nc.compile()
    return nc


def _get_nc():
    global _nc_cache
    if _nc_cache is None:
        _nc_cache = _build_nc()
    return _nc_cache


def _split_hi_lo(x64):
    """Split float64 values into bf16 hi + bf16 lo with hi + lo ~= x."""
    hi = x64.astype(BF16)
    lo = (x64 - hi.astype(np.float64)).astype(BF16)
    return hi, lo


def _prep_inputs(z, centroids):
    z = np.asarray(z, dtype=np.float32)
    c = np.asarray(centroids, dtype=np.float32)

    csqm1 = np.sum(c.astype(np.float64) ** 2, axis=1) - 1.0   # (512,)
    csq_hi, csq_lo = _split_hi_lo(csqm1)
    cx = np.empty((KX, NC_CLUST), dtype=BF16)
    cx[0] = csq_hi
    cx[1] = csq_lo
    cx[2] = BF16(1.0)
    cx[3] = BF16(1.0)

    zsq = np.sum(z.astype(np.float64) ** 2, axis=1)           # (8192,)
    zsq_hi, zsq_lo = _split_hi_lo(zsq)

    # z^T fp8: [h, b] -> [kc, j, p, b]
    zT8 = z.T.reshape(KC, 2, 128, BS).astype(FP8)
    # -2c^T fp8: [h2, kcl, j, p, c] -> [h2, p, kcl, j, c]
    ct8 = np.ascontiguousarray(
        (-2.0 * c.T).reshape(2, 4, 2, 128, NC_CLUST)
        .transpose(0, 3, 1, 2, 4)).astype(FP8).reshape(2, 128, 4 * 2 * NC_CLUST)

    in_maps = []
    for core in range(N_CORES):
        s = slice(core * B_CORE, (core + 1) * B_CORE)
        zx = np.empty((KX, B_CORE), dtype=BF16)
        zx[0] = BF16(1.0)
        zx[1] = BF16(1.0)
        zx[2] = zsq_hi[s]
        zx[3] = zsq_lo[s]
        # [kc, j, p, 8m, 128r] -> [m, p, kc, j, r]
        zc = np.ascontiguousarray(
            zT8[:, :, :, s].reshape(KC, 2, 128, M_TILES, 128)
            .transpose(3, 2, 0, 1, 4)).reshape(M_TILES, 128, KC * 2 * 128)
        in_maps.append({"zt": zc, "ct": ct8, "zx": zx, "cx": cx})
    return in_maps


def run(z, centroids, trace=False, trace_cores=None):
    """Run on the 8 NeuronCores. Returns (out, BassKernelResults)."""
    nc = _get_nc()
    in_maps = _prep_inputs(z, centroids)
    res = run_bass_kernel_spmd(
        nc, in_maps, list(range(N_CORES)),
        trace=trace, trace_cores=trace_cores,
    )
    qs, ps = [], []
    for core in range(N_CORES):
        qs.append(res.results[core]["q"].transpose(1, 0, 2)
                  .reshape(B_CORE, NC_CLUST))
        ps.append(res.results[core]["p"].transpose(1, 0, 2)
                  .reshape(B_CORE, NC_CLUST))
    q = np.concatenate(qs, axis=0).astype(np.float32)
    p = np.concatenate(ps, axis=0).astype(np.float32)
    out = np.stack([q, p])
    return out, res


def kernel(z, centroids):
    out, _ = run(z, centroids)
    return out
